# revision 3
# baseline (speedup 1.0000x reference)
"""Galerkin attention (ragged graph segments) on 8 Trainium2 NeuronCores — v2.

Math (per reference):
  qkv = x @ w_qkv.T ; split q,k,v -> [B, H, N, DH]
  k, v  <- LayerNorm over DH (eps=1e-6, affine)
  per graph g (sorted contiguous segments of N): ktv[g] = k_g^T v_g
  out_n = (q_n / size(g(n))) @ ktv[g(n)]
  y = out @ w_out.T + b_out

v2 design vs the fp32r baseline:
  * fp16 data path (x, W_qkv, k'/v', bd) -> 1 cycle/row matmuls at ALL free
    sizes, halved input DMA; all accumulation stays fp32 in PSUM.
  * LayerNorm stats via an ACT fp16 staging copy of kv plus two
    multi-group tensor_reduce calls (sum, sum of squares) and a handful of
    fused [P,16] ops instead of 16 per-head bn_stats/bn_aggr pairs; the
    elementwise work is spread over DVE + GPSIMD (GPSIMD is SBUF-only).
  * Augmented-Gram trick: X = [alpha*k | gamma], Y = [beta*v | delta] where
    LN(k) = alpha*k + gamma*1. ktv = P^T (X^T Y) P with P = [I; 1row], so
    the elementwise path needs only ONE broadcast multiply per side (via
    stride-0 free-dim APs) and the LN mean-shift becomes a rank-2 fixup
    accumulated into the 65x65 Gram with tiny matmuls per slot.
  * Odd heads store the aug column FIRST ([gamma | alpha*k]) and their Gram
    sits at PSUM partitions 63:128, so the pair-packed block-diagonal bd
    assembles with same-partition copies.
  * Phase 2 of each slot is emitted between the next slot's phase 1 and
    its Gram fixup, and the Gram matmuls trail the kv projection by one
    tile, so the in-order PE never stalls on the LayerNorm chain.
"""

import os
import sys

if "/opt/trn_rl_repo" not in sys.path:
    sys.path.insert(0, "/opt/trn_rl_repo")

import numpy as np

import concourse.bacc as bacc
import concourse.bass as bass
import concourse.mybir as mybir
import concourse.tile as tile
from concourse.bass_utils import run_bass_kernel_spmd

P = 128
B = 2
DIM = 512
HEADS = 8
DH = 64
DA = DH + 1                 # augmented head dim
INNER = HEADS * DH          # 512
R = 3 * INNER               # 1536
NCH = DIM // P              # 4 contraction chunks
NPAIRS = HEADS // 2         # 4 head pairs
EPS = 1e-6
N_CORES = 8
F32 = mybir.dt.float32
F32R = mybir.dt.float32r
F16 = mybir.dt.float16

_PROGRAM_CACHE: dict = {}


def _r(ap):
    return ap.bitcast(F32R)


def _apv(t, poff, pcnt, foff, dims):
    """Manual AP view of tile t: partition range [poff, poff+pcnt), free
    offset foff, free dims [[stride, n], ...] (strides in elements)."""
    a = t[:]
    ps = a.ap[0][0]
    return bass.AP(a.tensor, a.offset + poff * ps + foff,
                   [[ps, pcnt]] + [list(d) for d in dims])


def _groups(L, grp=4):
    """Split L tiles into chunks of <= grp, avoiding a trailing chunk of 1
    (which would drop fp32r matmuls below the 256 free-dim full-speed gate)."""
    if L <= grp:
        return [(0, L)]
    out = []
    t0 = 0
    while L - t0 > grp + 1:
        out.append((t0, grp))
        t0 += grp
    rem = L - t0
    if rem > grp:  # rem == grp + 1
        out.append((t0, grp - 1))
        out.append((t0 + grp - 1, 2))
    else:
        out.append((t0, rem))
    return out


# ---------------------------------------------------------------------------
# host-side planning (unchanged from baseline)
# ---------------------------------------------------------------------------

def _plan(batch, num_graphs, n_cores):
    batch = np.asarray(batch).astype(np.int64)
    G = int(num_graphs)
    counts = np.bincount(batch, minlength=G)[:G].astype(np.int64)
    starts = np.concatenate([[0], np.cumsum(counts)[:-1]])
    tiles_g = (counts + P - 1) // P

    S = (G + n_cores - 1) // n_cores
    order = np.argsort(-tiles_g, kind="stable")
    core_graphs = [[] for _ in range(n_cores)]
    core_load = [0] * n_cores
    for g in order:
        cands = [c for c in range(n_cores) if len(core_graphs[c]) < S]
        c = min(cands, key=lambda cc: (core_load[cc], cc))
        core_graphs[c].append(int(g))
        core_load[c] += int(tiles_g[g])
    for c in range(n_cores):
        core_graphs[c].sort(key=lambda g: -int(tiles_g[g]))
        while len(core_graphs[c]) < S:
            core_graphs[c].append(-1)

    Ls = []
    for s in range(S):
        L = max(
            int(tiles_g[core_graphs[c][s]]) if core_graphs[c][s] >= 0 else 0
            for c in range(n_cores)
        )
        Ls.append(max(L, 1))
    return counts, starts, core_graphs, Ls


def _pack_inputs(x, counts, starts, core_graphs, Ls, n_cores):
    T = sum(Ls)
    slot_off = np.concatenate([[0], np.cumsum(Ls)[:-1]])
    xT = np.ascontiguousarray(
        np.transpose(x, (0, 2, 1)).astype(np.float16))  # [B, DIM, N] fp16
    per_core = []
    for c in range(n_cores):
        xTp = np.zeros((B, DIM, T * P), np.float16)
        qsc = np.zeros((T * P,), np.float32)
        kvm = np.zeros((T * P,), np.float32)
        for s, g in enumerate(core_graphs[c]):
            if g < 0 or counts[g] == 0:
                continue
            n0, ng = int(starts[g]), int(counts[g])
            off = int(slot_off[s]) * P
            xTp[:, :, off:off + ng] = xT[:, :, n0:n0 + ng]
            qsc[off:off + ng] = 1.0 / ng
            kvm[off:off + ng] = 1.0
        per_core.append((xTp, qsc, kvm))
    return per_core, slot_off


# ---------------------------------------------------------------------------
# device program
# ---------------------------------------------------------------------------

def _build_program(T, Ls, n_cores, ln_general, bo_zero=False):
    from contextlib import ExitStack

    nc = bacc.Bacc("TRN2", target_bir_lowering=False, debug=False,
                   num_devices=n_cores)

    xT = nc.dram_tensor("xT", [B, DIM, T * P], F16, kind="ExternalInput")
    wq = nc.dram_tensor("wqkvT", [DIM, R], F16, kind="ExternalInput")
    wo = nc.dram_tensor("woutT", [INNER, DIM], F16, kind="ExternalInput")
    bo = nc.dram_tensor("bout", [DIM], F32, kind="ExternalInput")
    qsc = nc.dram_tensor("qsc", [T * P], F32, kind="ExternalInput")
    kvm = nc.dram_tensor("kvm", [T * P], F32, kind="ExternalInput")
    if ln_general:
        lnp = nc.dram_tensor("lnp", [4, DH], F32, kind="ExternalInput")
    out = nc.dram_tensor("out", [B, T * P, DIM], F16, kind="ExternalOutput")

    slot_off = [0]
    for L in Ls[:-1]:
        slot_off.append(slot_off[-1] + L)

    Sqrt = mybir.ActivationFunctionType.Sqrt
    mult = mybir.AluOpType.mult
    add = mybir.AluOpType.add
    sub = mybir.AluOpType.subtract

    with ExitStack() as ctx:
        tc = ctx.enter_context(tile.TileContext(nc))
        const = ctx.enter_context(tc.tile_pool(name="const", bufs=1))

        WQ = const.tile([P, NCH, R], F16, tag="WQ")
        nc.sync.dma_start(out=WQ[:], in_=wq.ap().rearrange("(k c) r -> c k r", c=P))
        WO = const.tile([P, NCH, DIM], F16, tag="WO")
        nc.sync.dma_start(out=WO[:], in_=wo.ap().rearrange("(k c) d -> c k d", c=P))
        QS = const.tile([P, T], F32, tag="QS")
        nc.sync.dma_start(out=QS[:], in_=qsc.ap().rearrange("(t p) -> p t", p=P))
        KM = const.tile([P, T], F32, tag="KM")
        nc.sync.dma_start(out=KM[:], in_=kvm.ap().rearrange("(t p) -> p t", p=P))
        EPSC = const.tile([P, 1], F32, tag="EPSC")
        nc.vector.memset(EPSC[:], EPS)
        ONESP = const.tile([P, DH], F16, tag="ONESP")
        nc.vector.memset(ONESP[:], 1.0)
        if not bo_zero:
            BOt = const.tile([P, DIM], F32, tag="BO")
            nc.sync.dma_start(out=BOt[:], in_=bo.ap().partition_broadcast(P))
        if ln_general:
            LNP = const.tile([P, 4, DH], F32, tag="LNP")
            nc.sync.dma_start(out=LNP[:], in_=lnp.ap().partition_broadcast(P))

        xpool = ctx.enter_context(tc.tile_pool(name="xp", bufs=3))
        kvsb = ctx.enter_context(tc.tile_pool(name="kvsb", bufs=3))
        stat = ctx.enter_context(tc.tile_pool(name="stat", bufs=3))
        qstash = ctx.enter_context(tc.tile_pool(name="qstash", bufs=2 * NPAIRS))
        fixp = ctx.enter_context(tc.tile_pool(name="fixp", bufs=2))
        bdsb = ctx.enter_context(tc.tile_pool(name="bd", bufs=2))
        ohsb = ctx.enter_context(tc.tile_pool(name="oh", bufs=2 * NPAIRS))
        outsb = ctx.enter_context(tc.tile_pool(name="outsb", bufs=3))

        kvps = ctx.enter_context(tc.tile_pool(name="kvps", bufs=2, space="PSUM"))
        pmix = ctx.enter_context(tc.tile_pool(name="pmix", bufs=2, space="PSUM"))
        gps = ctx.enter_context(tc.tile_pool(name="gps", bufs=1, space="PSUM"))

        def _phase2(job):
            b2, soff2, grps2, qts2, bd2 = job
            for (gt0, gw) in grps2:
                GW = gw * P
                ohs = []
                for p in range(NPAIRS):
                    oh = pmix.tile([P, GW], F32, tag="pm")
                    nc.tensor.matmul(
                        oh[:], lhsT=bd2[:, p, :],
                        rhs=qts2[p][:, gt0 * P:gt0 * P + GW],
                        start=True, stop=True)
                    os_ = ohsb.tile([P, GW], F16, name="ohs", tag="oh")
                    nc.scalar.copy(out=os_[:], in_=oh[:])
                    ohs.append(os_)

                for tl in range(gw):
                    t_ = gt0 + tl
                    ti = soff2 + t_
                    ops = pmix.tile([P, DIM], F32, tag="pm")
                    for p in range(NPAIRS):
                        nc.tensor.matmul(
                            ops[:],
                            lhsT=ohs[p][:, tl * P:(tl + 1) * P],
                            rhs=WO[:, p, :],
                            start=(p == 0), stop=(p == NPAIRS - 1))
                    ot = outsb.tile([P, DIM], F16, tag="ot")
                    if bo_zero:
                        nc.scalar.mul(ot[:], ops[:], QS[:, ti:ti + 1])
                    else:
                        nc.vector.scalar_tensor_tensor(
                            ot[:], ops[:], QS[:, ti:ti + 1], BOt[:],
                            op0=mult, op1=add)
                    nc.sync.dma_start(
                        out=out.ap()[b2, ti * P:(ti + 1) * P, :],
                        in_=ot[:])

        prev_job = None
        for b in range(B):
            for s, L in enumerate(Ls):
                soff = slot_off[s]
                # padded to [P, 4, 128] so each Gram owns a full 2KB
                # PSUM bank: matmul start=True clears has_written for the
                # WHOLE bank, so co-residents would corrupt each other.
                GE = gps.tile([P, NPAIRS, P], F32, tag="ge")
                GO = gps.tile([P, NPAIRS, P], F32, tag="go")
                # Gram init via matmul start=True pending-zero marking:
                # the first matmul of each partition-region marks its full
                # 2KB bank row pending-zero (per-partition), so each of the
                # three region chains below needs exactly one start.
                qts = [qstash.tile([P, L * P], F16, name=f"qts{i}", tag="qstash")
                       for i in range(NPAIRS)]
                grps = _groups(L)

                # aug-Gram accumulate, software-pipelined one tile behind
                # the kv matmul so the in-order PE never stalls on the LN
                # chain of the tile it just produced.
                #   even head 2p: [65, 65] at GE[0:65, p, :]
                #   odd head 2p+1: k'-rows [64, 65] at GO[64:128, p, :]
                #                  gamma-row [1, 65] at GO[32:33, p, :]
                # (PE tile bases must be 0/32/64, so the odd 65-row Gram is
                #  split: main block at base 64 so the bd copy stays
                #  same-partition, aug row parked at 32.)
                first = [True]

                def _emit_ktv(XT_, YT_):
                    st = first[0]
                    first[0] = False
                    for p in range(NPAIRS):
                        nc.tensor.matmul(
                            GE[0:DA, p, 0:DA],
                            lhsT=XT_[:, 0, p, :], rhs=YT_[:, 0, p, :],
                            start=(st and p == 0), stop=False,
                            skip_group_check=True)
                        nc.tensor.matmul(
                            GO[DH:P, p, 0:DA],
                            lhsT=XT_[:, 1, p, 1:DA], rhs=YT_[:, 1, p, :],
                            start=(st and p == 0), stop=False,
                            skip_group_check=True)
                        nc.tensor.matmul(
                            GO[32:33, p, 0:DA],
                            lhsT=XT_[:, 1, p, 0:1], rhs=YT_[:, 1, p, :],
                            start=(st and p == 0), stop=False,
                            skip_group_check=True)
                pend = [None]

                # ---- phase 1: qkv projection, LN stats, apply, aug-Gram ----
                for (gt0, gw) in grps:
                    GW = gw * P
                    n0 = (soff + gt0) * P

                    xt = xpool.tile([P, NCH, GW], F16, tag="xt")
                    nc.sync.dma_start(
                        out=xt[:],
                        in_=xT.ap()[b].rearrange("(k c) n -> c k n", c=P)[:, :, n0:n0 + GW],
                    )

                    # q^T pairs: stationary = W_q pair block, moving = x^T
                    for p in range(NPAIRS):
                        qtp = pmix.tile([P, GW], F32, tag="pm")
                        for k in range(NCH):
                            nc.tensor.matmul(
                                qtp[:],
                                lhsT=WQ[:, k, p * P:(p + 1) * P],
                                rhs=xt[:, k, :],
                                start=(k == 0), stop=(k == NCH - 1),
                            )
                        nc.scalar.copy(
                            out=qts[p][:, gt0 * P:gt0 * P + GW], in_=qtp[:])

                    for tl in range(gw):
                        t_ = gt0 + tl
                        ti = soff + t_

                        kv = kvps.tile([P, 2, HEADS, DH], F32, tag="kv")
                        for k in range(NCH):
                            lx = xt[:, k, tl * P:(tl + 1) * P]
                            nc.tensor.matmul(
                                kv[:, 0], lhsT=lx,
                                rhs=WQ[:, k, INNER:2 * INNER],
                                start=(k == 0), stop=(k == NCH - 1))
                            nc.tensor.matmul(
                                kv[:, 1], lhsT=lx,
                                rhs=WQ[:, k, 2 * INNER:R],
                                start=(k == 0), stop=(k == NCH - 1))
                        if pend[0] is not None:
                            _emit_ktv(*pend[0])
                            pend[0] = None

                        # Stage kv into SBUF fp16 (ACT) — frees the PSUM
                        # tile early and lets GPSIMD (SBUF-only) share the
                        # elementwise work. All LN math reads KVS.
                        KVS = kvsb.tile([P, 2, HEADS, DH], F16, tag="kvs")
                        nc.scalar.copy(out=KVS[:], in_=kv[:])

                        # LN stats: per-head sums and sums of squares via
                        # multi-group tensor_reduce (the wrapper keeps the
                        # window dim so head groups cannot merge).
                        SQ = kvsb.tile([P, 2, HEADS, DH], F16, tag="sq")
                        nc.vector.tensor_tensor(SQ[:], KVS[:], KVS[:], op=mult)
                        SU = stat.tile([P, 2, HEADS], F32, tag="su")
                        nc.vector.tensor_reduce(
                            SU[:], KVS[:], axis=mybir.AxisListType.X, op=add)
                        SS = stat.tile([P, 2, HEADS], F32, tag="ss")
                        nc.vector.tensor_reduce(
                            SS[:], SQ[:], axis=mybir.AxisListType.X, op=add)
                        # var*DH = SS - SU^2/DH ;  sd = sqrt(var + eps)
                        T1 = stat.tile([P, 2, HEADS], F32, tag="t1")
                        nc.gpsimd.tensor_tensor(T1[:], SU[:], SU[:], op=mult)
                        U2 = stat.tile([P, 2, HEADS], F32, tag="u2")
                        nc.vector.scalar_tensor_tensor(
                            U2[:], T1[:], -1.0 / DH, SS[:], op0=mult, op1=add)
                        SD = stat.tile([P, 2, HEADS], F32, tag="sd")
                        nc.scalar.activation(SD[:], U2[:], Sqrt,
                                             bias=EPSC[:, 0:1], scale=1.0 / DH)
                        RS = stat.tile([P, 2, HEADS], F32, tag="rs")
                        nc.vector.reciprocal(RS[:], SD[:])
                        # alpha/beta = rstd * mask ; gamma/delta = -mu * alpha
                        # ALGM layout [P, (al|gm), side, head]; fp16 copy for
                        # the applies / aug columns.
                        ALGM = stat.tile([P, 2, 2, HEADS], F32, tag="algm")
                        nc.vector.tensor_scalar(ALGM[:, 0], RS[:],
                                                KM[:, ti:ti + 1], None, op0=mult)
                        nc.vector.scalar_tensor_tensor(
                            ALGM[:, 1], SU[:], -1.0 / DH, ALGM[:, 0],
                            op0=mult, op1=mult)
                        AG16 = stat.tile([P, 2, 2, HEADS], F16, tag="ag16")
                        nc.gpsimd.tensor_copy(AG16[:], ALGM[:])

                        # apply: one broadcast multiply per side.
                        # XT layout [P, parity, pair, DA]:
                        #   even head 2p: cols 0:64 = alpha*k, col 64 = gamma
                        #   odd head 2p+1: col 0 = gamma, cols 1:65 = alpha*k
                        XT_ = kvsb.tile([P, 2, NPAIRS, DA], F16, tag="xtl")
                        YT_ = kvsb.tile([P, 2, NPAIRS, DA], F16, tag="ytl")
                        for side, dst, eng in ((0, XT_, nc.gpsimd),
                                               (1, YT_, nc.gpsimd)):
                            # out: parity stride 4*DA+1 (odd block shifted +1)
                            dstv = _apv(dst, 0, P, 0,
                                        [[NPAIRS * DA + 1, 2], [DA, NPAIRS], [1, DH]])
                            # in: (parity, pair, d) view of contiguous heads
                            srcv = _apv(KVS, 0, P, side * INNER,
                                        [[DH, 2], [2 * DH, NPAIRS], [1, DH]])
                            alv = _apv(AG16, 0, P, side * HEADS,
                                       [[1, 2], [2, NPAIRS], [0, DH]])
                            eng.tensor_tensor(dstv, srcv, alv, op=mult)
                            # aug column: even at col 64, odd at col 0 (+260)
                            colv = _apv(dst, 0, P, DH,
                                        [[NPAIRS * DA - DH, 2], [DA, NPAIRS]])
                            gmv = _apv(AG16, 0, P, 2 * HEADS + side * HEADS,
                                       [[1, 2], [2, NPAIRS]])
                            if ln_general:
                                nc.gpsimd.memset(colv, 0.0)
                            else:
                                nc.gpsimd.tensor_copy(colv, gmv)

                        if ln_general:
                            # full affine LN on X/Y directly (slow, correct):
                            #   z = w*(alpha*k + gamma) + b*mask
                            for side, dst in ((0, XT_), (1, YT_)):
                                dstv = _apv(dst, 0, P, 0,
                                            [[NPAIRS * DA + 1, 2], [DA, NPAIRS], [1, DH]])
                                wv = _apv(LNP, 0, P, 2 * side * DH,
                                          [[0, 2], [0, NPAIRS], [1, DH]])
                                nc.vector.tensor_tensor(dstv, dstv, wv, op=mult)
                                CV = kvsb.tile([P, 2, NPAIRS, DH], F32, tag="cv")
                                gmbc = _apv(ALGM, 0, P,
                                            2 * HEADS + side * HEADS,
                                            [[1, 2], [2, NPAIRS], [0, DH]])
                                nc.vector.tensor_tensor(CV[:], gmbc, wv, op=mult)
                                BM = stat.tile([P, DH], F32, tag="bm")
                                nc.vector.tensor_scalar(
                                    BM[:], LNP[:, 2 * side + 1], KM[:, ti:ti + 1],
                                    None, op0=mult)
                                bmbc = _apv(BM, 0, P, 0,
                                            [[0, 2], [0, NPAIRS], [1, DH]])
                                nc.vector.tensor_tensor(CV[:], CV[:], bmbc, op=add)
                                nc.vector.tensor_tensor(dstv, dstv, CV[:], op=add)

                        pend[0] = (XT_, YT_)

                if pend[0] is not None:
                    _emit_ktv(*pend[0])
                    pend[0] = None

                # previous slot's phase 2 fills the PE while this slot's
                # Gram fixup waits on the tail of the LN chain.
                if prev_job is not None:
                    _phase2(prev_job)
                    prev_job = None

                # ---- fixup: ktv_eff = G00 + 1 (x) (m10 + m11) + m01 (x) 1 ----
                # row term via a tiny ones-matmul accumulated into G; the
                # m01 column term rides the bd copy as a [P,1] scalar add.
                Rr_e = fixp.tile([P, NPAIRS, DH], F16, tag="rre")
                Rr_o = fixp.tile([P, NPAIRS, DH], F16, tag="rro")
                for p in range(NPAIRS):
                    nc.vector.tensor_scalar(
                        Rr_e[DH:DH + 1, p, :], GE[DH:DH + 1, p, 0:DH],
                        GE[DH:DH + 1, p, DA - 1:DA], None, op0=add)
                    nc.vector.tensor_scalar(
                        Rr_o[32:33, p, :], GO[32:33, p, 1:DA],
                        GO[32:33, p, 0:1], None, op0=add)
                for p in range(NPAIRS):
                    last = p == NPAIRS - 1
                    nc.tensor.matmul(
                        GE[0:DH, p, 0:DH], lhsT=ONESP[DH:DH + 1, :],
                        rhs=Rr_e[DH:DH + 1, p, :], start=False, stop=last,
                        skip_group_check=True)
                    nc.tensor.matmul(
                        GO[DH:P, p, 1:DA], lhsT=ONESP[32:33, :],
                        rhs=Rr_o[32:33, p, :], start=False, stop=last,
                        skip_group_check=True)

                # ---- block-diagonal ktv_eff for the pair-batched phase 2 ----
                bd = bdsb.tile([P, NPAIRS, P], F16, tag="bd")
                nc.gpsimd.memset(bd[:], 0.0)
                for p in range(NPAIRS):
                    nc.vector.tensor_scalar(
                        bd[0:DH, p, 0:DH], GE[0:DH, p, 0:DH],
                        GE[0:DH, p, DA - 1:DA], None, op0=add)
                    nc.vector.tensor_scalar(
                        bd[DH:P, p, DH:P], GO[DH:P, p, 1:DA],
                        GO[DH:P, p, 0:1], None, op0=add)

                prev_job = (b, soff, grps, qts, bd)

        if prev_job is not None:
            _phase2(prev_job)

    nc.compile()
    return nc


# ---------------------------------------------------------------------------
# entry point
# ---------------------------------------------------------------------------

def _run(x, w_qkv, ln1_w, ln1_b, ln2_w, ln2_b, w_out, b_out, batch,
         num_graphs, n_cores=N_CORES, trace=False):
    x = np.asarray(x)
    counts, starts, core_graphs, Ls = _plan(batch, num_graphs, n_cores)
    per_core, slot_off = _pack_inputs(x, counts, starts, core_graphs, Ls, n_cores)
    T = sum(Ls)

    ln1_w = np.asarray(ln1_w, np.float32)
    ln1_b = np.asarray(ln1_b, np.float32)
    ln2_w = np.asarray(ln2_w, np.float32)
    ln2_b = np.asarray(ln2_b, np.float32)
    ln_general = not (
        np.all(ln1_w == 1.0) and np.all(ln1_b == 0.0)
        and np.all(ln2_w == 1.0) and np.all(ln2_b == 0.0)
    )

    bout_np = np.asarray(b_out, np.float32)
    bo_zero = bool(np.all(bout_np == 0.0))
    key = (T, tuple(Ls), n_cores, ln_general, bo_zero)
    nc = _PROGRAM_CACHE.get(key)
    if nc is None:
        nc = _build_program(T, tuple(Ls), n_cores, ln_general, bo_zero)
        _PROGRAM_CACHE[key] = nc

    wqkvT = np.ascontiguousarray(np.asarray(w_qkv, np.float32).T.astype(np.float16))
    woutT = np.ascontiguousarray(np.asarray(w_out, np.float32).T.astype(np.float16))
    bout = np.ascontiguousarray(bout_np)
    lnp = np.stack([ln1_w, ln1_b, ln2_w, ln2_b])

    in_maps = []
    for c in range(n_cores):
        xTp, qscv, kvmv = per_core[c]
        m = {"xT": xTp, "wqkvT": wqkvT, "woutT": woutT, "bout": bout,
             "qsc": qscv, "kvm": kvmv}
        if ln_general:
            m["lnp"] = lnp
        in_maps.append(m)

    res = run_bass_kernel_spmd(nc, in_maps, list(range(n_cores)), trace=trace)

    N = x.shape[1]
    y = np.empty((B, N, DIM), np.float32)
    for c in range(n_cores):
        oc = res.results[c]["out"]
        for s, g in enumerate(core_graphs[c]):
            if g < 0 or counts[g] == 0:
                continue
            n0, ng = int(starts[g]), int(counts[g])
            off = int(slot_off[s]) * P
            y[:, n0:n0 + ng, :] = oc[:, off:off + ng, :].astype(np.float32)
    return y, res


def kernel(**inputs):
    trace = os.environ.get("GALERKIN_TRACE") == "1"
    y, _ = _run(
        inputs["x"], inputs["w_qkv"], inputs["ln1_w"], inputs["ln1_b"],
        inputs["ln2_w"], inputs["ln2_b"], inputs["w_out"], inputs["b_out"],
        inputs["batch"], inputs["num_graphs"], trace=trace,
    )
    return y


# revision 4
# speedup vs baseline: 3.7870x; 3.7870x over previous
"""Galerkin attention (ragged graph segments) on 8 Trainium2 NeuronCores — v2.

Math (per reference):
  qkv = x @ w_qkv.T ; split q,k,v -> [B, H, N, DH]
  k, v  <- LayerNorm over DH (eps=1e-6, affine)
  per graph g (sorted contiguous segments of N): ktv[g] = k_g^T v_g
  out_n = (q_n / size(g(n))) @ ktv[g(n)]
  y = out @ w_out.T + b_out

v2 design vs the fp32r baseline:
  * fp16 data path (x, W_qkv, k'/v', bd) -> 1 cycle/row matmuls at ALL free
    sizes, halved input DMA; all accumulation stays fp32 in PSUM.
  * LayerNorm stats via an ACT fp16 staging copy of kv plus two
    multi-group tensor_reduce calls (sum, sum of squares) and a handful of
    fused [P,16] ops instead of 16 per-head bn_stats/bn_aggr pairs; the
    elementwise work is spread over DVE + GPSIMD (GPSIMD is SBUF-only).
  * Augmented-Gram trick: X = [alpha*k | gamma], Y = [beta*v | delta] where
    LN(k) = alpha*k + gamma*1. ktv = P^T (X^T Y) P with P = [I; 1row], so
    the elementwise path needs only ONE broadcast multiply per side (via
    stride-0 free-dim APs) and the LN mean-shift becomes a rank-2 fixup
    accumulated into the 65x65 Gram with tiny matmuls per slot.
  * Odd heads store the aug column FIRST ([gamma | alpha*k]) and their Gram
    sits at PSUM partitions 63:128, so the pair-packed block-diagonal bd
    assembles with same-partition copies.
  * Phase 2 of each slot is emitted between the next slot's phase 1 and
    its Gram fixup, and the Gram matmuls trail the kv projection by one
    tile, so the in-order PE never stalls on the LayerNorm chain.
"""

import os
import sys

if "/opt/trn_rl_repo" not in sys.path:
    sys.path.insert(0, "/opt/trn_rl_repo")

import numpy as np

import concourse.bacc as bacc
import concourse.bass as bass
import concourse.mybir as mybir
import concourse.tile as tile
from concourse.bass_utils import run_bass_kernel_spmd

P = 128
B = 2
DIM = 512
HEADS = 8
DH = 64
DA = DH + 1                 # augmented head dim
INNER = HEADS * DH          # 512
R = 3 * INNER               # 1536
NCH = DIM // P              # 4 contraction chunks
NPAIRS = HEADS // 2         # 4 head pairs
EPS = 1e-6
N_CORES = 8
F32 = mybir.dt.float32
F32R = mybir.dt.float32r
F16 = mybir.dt.float16

_PROGRAM_CACHE: dict = {}


def _r(ap):
    return ap.bitcast(F32R)


def _apv(t, poff, pcnt, foff, dims):
    """Manual AP view of tile t: partition range [poff, poff+pcnt), free
    offset foff, free dims [[stride, n], ...] (strides in elements)."""
    a = t[:]
    ps = a.ap[0][0]
    return bass.AP(a.tensor, a.offset + poff * ps + foff,
                   [[ps, pcnt]] + [list(d) for d in dims])


def _groups(L, grp=4):
    """Split L tiles into chunks of <= grp, avoiding a trailing chunk of 1
    (which would drop fp32r matmuls below the 256 free-dim full-speed gate)."""
    if L <= grp:
        return [(0, L)]
    out = []
    t0 = 0
    while L - t0 > grp + 1:
        out.append((t0, grp))
        t0 += grp
    rem = L - t0
    if rem > grp:  # rem == grp + 1
        out.append((t0, grp - 1))
        out.append((t0 + grp - 1, 2))
    else:
        out.append((t0, rem))
    return out


# ---------------------------------------------------------------------------
# host-side planning (unchanged from baseline)
# ---------------------------------------------------------------------------

def _plan(batch, num_graphs, n_cores):
    batch = np.asarray(batch).astype(np.int64)
    G = int(num_graphs)
    counts = np.bincount(batch, minlength=G)[:G].astype(np.int64)
    starts = np.concatenate([[0], np.cumsum(counts)[:-1]])
    tiles_g = (counts + P - 1) // P

    S = (G + n_cores - 1) // n_cores
    order = np.argsort(-tiles_g, kind="stable")
    core_graphs = [[] for _ in range(n_cores)]
    core_load = [0] * n_cores
    for g in order:
        cands = [c for c in range(n_cores) if len(core_graphs[c]) < S]
        c = min(cands, key=lambda cc: (core_load[cc], cc))
        core_graphs[c].append(int(g))
        core_load[c] += int(tiles_g[g])
    for c in range(n_cores):
        core_graphs[c].sort(key=lambda g: -int(tiles_g[g]))
        while len(core_graphs[c]) < S:
            core_graphs[c].append(-1)

    Ls = []
    for s in range(S):
        L = max(
            int(tiles_g[core_graphs[c][s]]) if core_graphs[c][s] >= 0 else 0
            for c in range(n_cores)
        )
        Ls.append(max(L, 1))
    return counts, starts, core_graphs, Ls


def _pack_inputs(x, counts, starts, core_graphs, Ls, n_cores):
    T = sum(Ls)
    slot_off = np.concatenate([[0], np.cumsum(Ls)[:-1]])
    xT = np.ascontiguousarray(
        np.transpose(x, (0, 2, 1)).astype(np.float16))  # [B, DIM, N] fp16
    per_core = []
    for c in range(n_cores):
        xTp = np.zeros((B, DIM, T * P), np.float16)
        qsc = np.zeros((T * P,), np.float32)
        kvm = np.zeros((T * P,), np.float32)
        for s, g in enumerate(core_graphs[c]):
            if g < 0 or counts[g] == 0:
                continue
            n0, ng = int(starts[g]), int(counts[g])
            off = int(slot_off[s]) * P
            xTp[:, :, off:off + ng] = xT[:, :, n0:n0 + ng]
            qsc[off:off + ng] = 1.0 / ng
            kvm[off:off + ng] = 1.0
        per_core.append((xTp, qsc, kvm))
    return per_core, slot_off


# ---------------------------------------------------------------------------
# device program
# ---------------------------------------------------------------------------

def _build_program(T, Ls, n_cores, ln_general, bo_zero=False, repeat=1):
    from contextlib import ExitStack

    nc = bacc.Bacc("TRN2", target_bir_lowering=False, debug=False,
                   num_devices=n_cores)

    xT = nc.dram_tensor("xT", [B, DIM, T * P], F16, kind="ExternalInput")
    wq = nc.dram_tensor("wqkvT", [DIM, R], F16, kind="ExternalInput")
    wo = nc.dram_tensor("woutT", [INNER, DIM], F16, kind="ExternalInput")
    bo = nc.dram_tensor("bout", [DIM], F32, kind="ExternalInput")
    qsc = nc.dram_tensor("qsc", [T * P], F32, kind="ExternalInput")
    kvm = nc.dram_tensor("kvm", [T * P], F32, kind="ExternalInput")
    if ln_general:
        lnp = nc.dram_tensor("lnp", [4, DH], F32, kind="ExternalInput")
    out = nc.dram_tensor("out", [B, T * P, DIM], F16, kind="ExternalOutput")

    slot_off = [0]
    for L in Ls[:-1]:
        slot_off.append(slot_off[-1] + L)

    Sqrt = mybir.ActivationFunctionType.Sqrt
    mult = mybir.AluOpType.mult
    add = mybir.AluOpType.add
    sub = mybir.AluOpType.subtract

    with ExitStack() as ctx:
        tc = ctx.enter_context(tile.TileContext(nc))
        const = ctx.enter_context(tc.tile_pool(name="const", bufs=1))

        WQ = const.tile([P, NCH, R], F16, tag="WQ")
        nc.sync.dma_start(out=WQ[:], in_=wq.ap().rearrange("(k c) r -> c k r", c=P))
        WO = const.tile([P, NCH, DIM], F16, tag="WO")
        nc.sync.dma_start(out=WO[:], in_=wo.ap().rearrange("(k c) d -> c k d", c=P))
        QS = const.tile([P, T], F32, tag="QS")
        nc.sync.dma_start(out=QS[:], in_=qsc.ap().rearrange("(t p) -> p t", p=P))
        KM = const.tile([P, T], F32, tag="KM")
        nc.sync.dma_start(out=KM[:], in_=kvm.ap().rearrange("(t p) -> p t", p=P))
        EPSC = const.tile([P, 1], F32, tag="EPSC")
        nc.vector.memset(EPSC[:], EPS)
        ONESP = const.tile([P, DH], F16, tag="ONESP")
        nc.vector.memset(ONESP[:], 1.0)
        if not bo_zero:
            BOt = const.tile([P, DIM], F32, tag="BO")
            nc.sync.dma_start(out=BOt[:], in_=bo.ap().partition_broadcast(P))
        if ln_general:
            LNP = const.tile([P, 4, DH], F32, tag="LNP")
            nc.sync.dma_start(out=LNP[:], in_=lnp.ap().partition_broadcast(P))

        xpool = ctx.enter_context(tc.tile_pool(name="xp", bufs=3))
        kvsb = ctx.enter_context(tc.tile_pool(name="kvsb", bufs=3))
        stat = ctx.enter_context(tc.tile_pool(name="stat", bufs=3))
        qstash = ctx.enter_context(tc.tile_pool(name="qstash", bufs=2 * NPAIRS))
        fixp = ctx.enter_context(tc.tile_pool(name="fixp", bufs=2))
        bdsb = ctx.enter_context(tc.tile_pool(name="bd", bufs=2))
        ohsb = ctx.enter_context(tc.tile_pool(name="oh", bufs=2 * NPAIRS))
        outsb = ctx.enter_context(tc.tile_pool(name="outsb", bufs=3))

        kvps = ctx.enter_context(tc.tile_pool(name="kvps", bufs=2, space="PSUM"))
        pmix = ctx.enter_context(tc.tile_pool(name="pmix", bufs=2, space="PSUM"))
        gps = ctx.enter_context(tc.tile_pool(name="gps", bufs=1, space="PSUM"))

        def _phase2(job):
            b2, soff2, grps2, qts2, bd2 = job
            for (gt0, gw) in grps2:
                GW = gw * P
                ohs = []
                for p in range(NPAIRS):
                    oh = pmix.tile([P, GW], F32, tag="pm")
                    nc.tensor.matmul(
                        oh[:], lhsT=bd2[:, p, :],
                        rhs=qts2[p][:, gt0 * P:gt0 * P + GW],
                        start=True, stop=True)
                    os_ = ohsb.tile([P, GW], F16, name="ohs", tag="oh")
                    nc.scalar.copy(out=os_[:], in_=oh[:])
                    ohs.append(os_)

                for tl in range(gw):
                    t_ = gt0 + tl
                    ti = soff2 + t_
                    ops = pmix.tile([P, DIM], F32, tag="pm")
                    for p in range(NPAIRS):
                        nc.tensor.matmul(
                            ops[:],
                            lhsT=ohs[p][:, tl * P:(tl + 1) * P],
                            rhs=WO[:, p, :],
                            start=(p == 0), stop=(p == NPAIRS - 1))
                    ot = outsb.tile([P, DIM], F16, tag="ot")
                    if bo_zero:
                        nc.scalar.mul(ot[:], ops[:], QS[:, ti:ti + 1])
                    else:
                        nc.vector.scalar_tensor_tensor(
                            ot[:], ops[:], QS[:, ti:ti + 1], BOt[:],
                            op0=mult, op1=add)
                    nc.sync.dma_start(
                        out=out.ap()[b2, ti * P:(ti + 1) * P, :],
                        in_=ot[:])

        prev_job = None
        for _rep in range(repeat):
          for b in range(B):
            for s, L in enumerate(Ls):
                soff = slot_off[s]
                # padded to [P, 4, 128] so each Gram owns a full 2KB
                # PSUM bank: matmul start=True clears has_written for the
                # WHOLE bank, so co-residents would corrupt each other.
                GE = gps.tile([P, NPAIRS, P], F32, tag="ge")
                GO = gps.tile([P, NPAIRS, P], F32, tag="go")
                # Gram init via matmul start=True pending-zero marking:
                # the first matmul of each partition-region marks its full
                # 2KB bank row pending-zero (per-partition), so each of the
                # three region chains below needs exactly one start.
                qts = [qstash.tile([P, L * P], F16, name=f"qts{i}", tag="qstash")
                       for i in range(NPAIRS)]
                grps = _groups(L)

                # aug-Gram accumulate, software-pipelined one tile behind
                # the kv matmul so the in-order PE never stalls on the LN
                # chain of the tile it just produced.
                #   even head 2p: [65, 65] at GE[0:65, p, :]
                #   odd head 2p+1: k'-rows [64, 65] at GO[64:128, p, :]
                #                  gamma-row [1, 65] at GO[32:33, p, :]
                # (PE tile bases must be 0/32/64, so the odd 65-row Gram is
                #  split: main block at base 64 so the bd copy stays
                #  same-partition, aug row parked at 32.)
                first = [True]

                def _emit_ktv(XT_, YT_):
                    st = first[0]
                    first[0] = False
                    for p in range(NPAIRS):
                        nc.tensor.matmul(
                            GE[0:DA, p, 0:DA],
                            lhsT=XT_[:, 0, p, :], rhs=YT_[:, 0, p, :],
                            start=(st and p == 0), stop=False,
                            skip_group_check=True)
                        nc.tensor.matmul(
                            GO[DH:P, p, 0:DA],
                            lhsT=XT_[:, 1, p, 1:DA], rhs=YT_[:, 1, p, :],
                            start=(st and p == 0), stop=False,
                            skip_group_check=True)
                        nc.tensor.matmul(
                            GO[32:33, p, 0:DA],
                            lhsT=XT_[:, 1, p, 0:1], rhs=YT_[:, 1, p, :],
                            start=(st and p == 0), stop=False,
                            skip_group_check=True)
                pend = [None]

                # ---- phase 1: qkv projection, LN stats, apply, aug-Gram ----
                for (gt0, gw) in grps:
                    GW = gw * P
                    n0 = (soff + gt0) * P

                    xt = xpool.tile([P, NCH, GW], F16, tag="xt")
                    nc.sync.dma_start(
                        out=xt[:],
                        in_=xT.ap()[b].rearrange("(k c) n -> c k n", c=P)[:, :, n0:n0 + GW],
                    )

                    # q^T pairs: stationary = W_q pair block, moving = x^T
                    for p in range(NPAIRS):
                        qtp = pmix.tile([P, GW], F32, tag="pm")
                        for k in range(NCH):
                            nc.tensor.matmul(
                                qtp[:],
                                lhsT=WQ[:, k, p * P:(p + 1) * P],
                                rhs=xt[:, k, :],
                                start=(k == 0), stop=(k == NCH - 1),
                            )
                        nc.scalar.copy(
                            out=qts[p][:, gt0 * P:gt0 * P + GW], in_=qtp[:])

                    for tl in range(gw):
                        t_ = gt0 + tl
                        ti = soff + t_

                        kv = kvps.tile([P, 2, HEADS, DH], F32, tag="kv")
                        for k in range(NCH):
                            lx = xt[:, k, tl * P:(tl + 1) * P]
                            nc.tensor.matmul(
                                kv[:, 0], lhsT=lx,
                                rhs=WQ[:, k, INNER:2 * INNER],
                                start=(k == 0), stop=(k == NCH - 1))
                            nc.tensor.matmul(
                                kv[:, 1], lhsT=lx,
                                rhs=WQ[:, k, 2 * INNER:R],
                                start=(k == 0), stop=(k == NCH - 1))
                        if pend[0] is not None:
                            _emit_ktv(*pend[0])
                            pend[0] = None

                        # Stage kv into SBUF fp16 (ACT) — frees the PSUM
                        # tile early and lets GPSIMD (SBUF-only) share the
                        # elementwise work. All LN math reads KVS.
                        KVS = kvsb.tile([P, 2, HEADS, DH], F16, tag="kvs")
                        nc.scalar.copy(out=KVS[:], in_=kv[:])

                        # LN stats: per-head sums and sums of squares via
                        # multi-group tensor_reduce (the wrapper keeps the
                        # window dim so head groups cannot merge).
                        SQ = kvsb.tile([P, 2, HEADS, DH], F16, tag="sq")
                        nc.vector.tensor_tensor(SQ[:], KVS[:], KVS[:], op=mult)
                        SU = stat.tile([P, 2, HEADS], F32, tag="su")
                        nc.vector.tensor_reduce(
                            SU[:], KVS[:], axis=mybir.AxisListType.X, op=add)
                        SS = stat.tile([P, 2, HEADS], F32, tag="ss")
                        nc.vector.tensor_reduce(
                            SS[:], SQ[:], axis=mybir.AxisListType.X, op=add)
                        # var*DH = SS - SU^2/DH ;  sd = sqrt(var + eps)
                        T1 = stat.tile([P, 2, HEADS], F32, tag="t1")
                        nc.gpsimd.tensor_tensor(T1[:], SU[:], SU[:], op=mult)
                        U2 = stat.tile([P, 2, HEADS], F32, tag="u2")
                        nc.vector.scalar_tensor_tensor(
                            U2[:], T1[:], -1.0 / DH, SS[:], op0=mult, op1=add)
                        SD = stat.tile([P, 2, HEADS], F32, tag="sd")
                        nc.scalar.activation(SD[:], U2[:], Sqrt,
                                             bias=EPSC[:, 0:1], scale=1.0 / DH)
                        RS = stat.tile([P, 2, HEADS], F32, tag="rs")
                        nc.vector.reciprocal(RS[:], SD[:])
                        # alpha/beta = rstd * mask ; gamma/delta = -mu * alpha
                        # ALGM layout [P, (al|gm), side, head]; fp16 copy for
                        # the applies / aug columns.
                        ALGM = stat.tile([P, 2, 2, HEADS], F32, tag="algm")
                        nc.vector.tensor_scalar(ALGM[:, 0], RS[:],
                                                KM[:, ti:ti + 1], None, op0=mult)
                        nc.vector.scalar_tensor_tensor(
                            ALGM[:, 1], SU[:], -1.0 / DH, ALGM[:, 0],
                            op0=mult, op1=mult)
                        AG16 = stat.tile([P, 2, 2, HEADS], F16, tag="ag16")
                        nc.gpsimd.tensor_copy(AG16[:], ALGM[:])

                        # apply: one broadcast multiply per side.
                        # XT layout [P, parity, pair, DA]:
                        #   even head 2p: cols 0:64 = alpha*k, col 64 = gamma
                        #   odd head 2p+1: col 0 = gamma, cols 1:65 = alpha*k
                        XT_ = kvsb.tile([P, 2, NPAIRS, DA], F16, tag="xtl")
                        YT_ = kvsb.tile([P, 2, NPAIRS, DA], F16, tag="ytl")
                        for side, dst, eng in ((0, XT_, nc.gpsimd),
                                               (1, YT_, nc.gpsimd)):
                            # out: parity stride 4*DA+1 (odd block shifted +1)
                            dstv = _apv(dst, 0, P, 0,
                                        [[NPAIRS * DA + 1, 2], [DA, NPAIRS], [1, DH]])
                            # in: (parity, pair, d) view of contiguous heads
                            srcv = _apv(KVS, 0, P, side * INNER,
                                        [[DH, 2], [2 * DH, NPAIRS], [1, DH]])
                            alv = _apv(AG16, 0, P, side * HEADS,
                                       [[1, 2], [2, NPAIRS], [0, DH]])
                            eng.tensor_tensor(dstv, srcv, alv, op=mult)
                            # aug column: even at col 64, odd at col 0 (+260)
                            colv = _apv(dst, 0, P, DH,
                                        [[NPAIRS * DA - DH, 2], [DA, NPAIRS]])
                            gmv = _apv(AG16, 0, P, 2 * HEADS + side * HEADS,
                                       [[1, 2], [2, NPAIRS]])
                            if ln_general:
                                nc.gpsimd.memset(colv, 0.0)
                            else:
                                nc.gpsimd.tensor_copy(colv, gmv)

                        if ln_general:
                            # full affine LN on X/Y directly (slow, correct):
                            #   z = w*(alpha*k + gamma) + b*mask
                            for side, dst in ((0, XT_), (1, YT_)):
                                dstv = _apv(dst, 0, P, 0,
                                            [[NPAIRS * DA + 1, 2], [DA, NPAIRS], [1, DH]])
                                wv = _apv(LNP, 0, P, 2 * side * DH,
                                          [[0, 2], [0, NPAIRS], [1, DH]])
                                nc.vector.tensor_tensor(dstv, dstv, wv, op=mult)
                                CV = kvsb.tile([P, 2, NPAIRS, DH], F32, tag="cv")
                                gmbc = _apv(ALGM, 0, P,
                                            2 * HEADS + side * HEADS,
                                            [[1, 2], [2, NPAIRS], [0, DH]])
                                nc.vector.tensor_tensor(CV[:], gmbc, wv, op=mult)
                                BM = stat.tile([P, DH], F32, tag="bm")
                                nc.vector.tensor_scalar(
                                    BM[:], LNP[:, 2 * side + 1], KM[:, ti:ti + 1],
                                    None, op0=mult)
                                bmbc = _apv(BM, 0, P, 0,
                                            [[0, 2], [0, NPAIRS], [1, DH]])
                                nc.vector.tensor_tensor(CV[:], CV[:], bmbc, op=add)
                                nc.vector.tensor_tensor(dstv, dstv, CV[:], op=add)

                        pend[0] = (XT_, YT_)

                if pend[0] is not None:
                    _emit_ktv(*pend[0])
                    pend[0] = None

                # previous slot's phase 2 fills the PE while this slot's
                # Gram fixup waits on the tail of the LN chain.
                if prev_job is not None:
                    _phase2(prev_job)
                    prev_job = None

                # ---- fixup: ktv_eff = G00 + 1 (x) (m10 + m11) + m01 (x) 1 ----
                # row term via a tiny ones-matmul accumulated into G; the
                # m01 column term rides the bd copy as a [P,1] scalar add.
                Rr_e = fixp.tile([P, NPAIRS, DH], F16, tag="rre")
                Rr_o = fixp.tile([P, NPAIRS, DH], F16, tag="rro")
                for p in range(NPAIRS):
                    nc.vector.tensor_scalar(
                        Rr_e[DH:DH + 1, p, :], GE[DH:DH + 1, p, 0:DH],
                        GE[DH:DH + 1, p, DA - 1:DA], None, op0=add)
                    nc.vector.tensor_scalar(
                        Rr_o[32:33, p, :], GO[32:33, p, 1:DA],
                        GO[32:33, p, 0:1], None, op0=add)
                for p in range(NPAIRS):
                    last = p == NPAIRS - 1
                    nc.tensor.matmul(
                        GE[0:DH, p, 0:DH], lhsT=ONESP[DH:DH + 1, :],
                        rhs=Rr_e[DH:DH + 1, p, :], start=False, stop=last,
                        skip_group_check=True)
                    nc.tensor.matmul(
                        GO[DH:P, p, 1:DA], lhsT=ONESP[32:33, :],
                        rhs=Rr_o[32:33, p, :], start=False, stop=last,
                        skip_group_check=True)

                # ---- block-diagonal ktv_eff for the pair-batched phase 2 ----
                bd = bdsb.tile([P, NPAIRS, P], F16, tag="bd")
                nc.gpsimd.memset(bd[:], 0.0)
                for p in range(NPAIRS):
                    nc.vector.tensor_scalar(
                        bd[0:DH, p, 0:DH], GE[0:DH, p, 0:DH],
                        GE[0:DH, p, DA - 1:DA], None, op0=add)
                    nc.vector.tensor_scalar(
                        bd[DH:P, p, DH:P], GO[DH:P, p, 1:DA],
                        GO[DH:P, p, 0:1], None, op0=add)

                prev_job = (b, soff, grps, qts, bd)

        if prev_job is not None:
            _phase2(prev_job)

    nc.compile()
    return nc


# ---------------------------------------------------------------------------
# entry point
# ---------------------------------------------------------------------------

def _run(x, w_qkv, ln1_w, ln1_b, ln2_w, ln2_b, w_out, b_out, batch,
         num_graphs, n_cores=N_CORES, trace=False):
    x = np.asarray(x)
    counts, starts, core_graphs, Ls = _plan(batch, num_graphs, n_cores)
    per_core, slot_off = _pack_inputs(x, counts, starts, core_graphs, Ls, n_cores)
    T = sum(Ls)

    ln1_w = np.asarray(ln1_w, np.float32)
    ln1_b = np.asarray(ln1_b, np.float32)
    ln2_w = np.asarray(ln2_w, np.float32)
    ln2_b = np.asarray(ln2_b, np.float32)
    ln_general = not (
        np.all(ln1_w == 1.0) and np.all(ln1_b == 0.0)
        and np.all(ln2_w == 1.0) and np.all(ln2_b == 0.0)
    )

    bout_np = np.asarray(b_out, np.float32)
    bo_zero = bool(np.all(bout_np == 0.0))
    key = (T, tuple(Ls), n_cores, ln_general, bo_zero)
    nc = _PROGRAM_CACHE.get(key)
    if nc is None:
        nc = _build_program(T, tuple(Ls), n_cores, ln_general, bo_zero)
        _PROGRAM_CACHE[key] = nc

    wqkvT = np.ascontiguousarray(np.asarray(w_qkv, np.float32).T.astype(np.float16))
    woutT = np.ascontiguousarray(np.asarray(w_out, np.float32).T.astype(np.float16))
    bout = np.ascontiguousarray(bout_np)
    lnp = np.stack([ln1_w, ln1_b, ln2_w, ln2_b])

    in_maps = []
    for c in range(n_cores):
        xTp, qscv, kvmv = per_core[c]
        m = {"xT": xTp, "wqkvT": wqkvT, "woutT": woutT, "bout": bout,
             "qsc": qscv, "kvm": kvmv}
        if ln_general:
            m["lnp"] = lnp
        in_maps.append(m)

    res = run_bass_kernel_spmd(nc, in_maps, list(range(n_cores)), trace=trace)

    N = x.shape[1]
    y = np.empty((B, N, DIM), np.float32)
    for c in range(n_cores):
        oc = res.results[c]["out"]
        for s, g in enumerate(core_graphs[c]):
            if g < 0 or counts[g] == 0:
                continue
            n0, ng = int(starts[g]), int(counts[g])
            off = int(slot_off[s]) * P
            y[:, n0:n0 + ng, :] = oc[:, off:off + ng, :].astype(np.float32)
    return y, res


def kernel(**inputs):
    trace = os.environ.get("GALERKIN_TRACE") == "1"
    y, _ = _run(
        inputs["x"], inputs["w_qkv"], inputs["ln1_w"], inputs["ln1_b"],
        inputs["ln2_w"], inputs["ln2_b"], inputs["w_out"], inputs["b_out"],
        inputs["batch"], inputs["num_graphs"], trace=trace,
    )
    return y


# revision 5
# speedup vs baseline: 4.4940x; 1.1867x over previous
"""Galerkin attention (ragged graph segments) on 8 Trainium2 NeuronCores — v2.

Math (per reference):
  qkv = x @ w_qkv.T ; split q,k,v -> [B, H, N, DH]
  k, v  <- LayerNorm over DH (eps=1e-6, affine)
  per graph g (sorted contiguous segments of N): ktv[g] = k_g^T v_g
  out_n = (q_n / size(g(n))) @ ktv[g(n)]
  y = out @ w_out.T + b_out

v2 design vs the fp32r baseline:
  * fp16 data path (x, W_qkv, k'/v', bd) -> 1 cycle/row matmuls at ALL free
    sizes, halved input DMA; all accumulation stays fp32 in PSUM.
  * LayerNorm stats via an ACT fp16 staging copy of kv plus two
    multi-group tensor_reduce calls (sum, sum of squares) and a handful of
    fused [P,16] ops instead of 16 per-head bn_stats/bn_aggr pairs; the
    elementwise work is spread over DVE + GPSIMD (GPSIMD is SBUF-only).
  * LN apply as two broadcast ops per side (alpha*k + gamma via stride-0
    free-dim APs); per-head 64x64 Grams pair-pack into one PSUM bank with
    even heads at partitions 0:64 and odd heads at 64:128 (matmul output
    base picks the partition block), so the block-diagonal bd assembles
    with two batched same-partition copies and no fixup pass.
  * Phase 2 of each slot is emitted between the next slot's phase 1 and
    its Gram fixup, and the Gram matmuls trail the kv projection by one
    tile, so the in-order PE never stalls on the LayerNorm chain.
"""

import os
import sys

if "/opt/trn_rl_repo" not in sys.path:
    sys.path.insert(0, "/opt/trn_rl_repo")

import numpy as np

import concourse.bacc as bacc
import concourse.bass as bass
import concourse.mybir as mybir
import concourse.tile as tile
from concourse.bass_utils import run_bass_kernel_spmd

P = 128
B = 2
DIM = 512
HEADS = 8
DH = 64
DA = DH + 1                 # augmented head dim
INNER = HEADS * DH          # 512
R = 3 * INNER               # 1536
NCH = DIM // P              # 4 contraction chunks
NPAIRS = HEADS // 2         # 4 head pairs
EPS = 1e-6
N_CORES = 8
F32 = mybir.dt.float32
F32R = mybir.dt.float32r
F16 = mybir.dt.float16

_PROGRAM_CACHE: dict = {}


def _r(ap):
    return ap.bitcast(F32R)


def _apv(t, poff, pcnt, foff, dims):
    """Manual AP view of tile t: partition range [poff, poff+pcnt), free
    offset foff, free dims [[stride, n], ...] (strides in elements)."""
    a = t[:]
    ps = a.ap[0][0]
    return bass.AP(a.tensor, a.offset + poff * ps + foff,
                   [[ps, pcnt]] + [list(d) for d in dims])


def _groups(L, grp=4):
    """Split L tiles into chunks of <= grp, avoiding a trailing chunk of 1
    (which would drop fp32r matmuls below the 256 free-dim full-speed gate)."""
    if L <= grp:
        return [(0, L)]
    out = []
    t0 = 0
    while L - t0 > grp + 1:
        out.append((t0, grp))
        t0 += grp
    rem = L - t0
    if rem > grp:  # rem == grp + 1
        out.append((t0, grp - 1))
        out.append((t0 + grp - 1, 2))
    else:
        out.append((t0, rem))
    return out


# ---------------------------------------------------------------------------
# host-side planning (unchanged from baseline)
# ---------------------------------------------------------------------------

def _plan(batch, num_graphs, n_cores):
    batch = np.asarray(batch).astype(np.int64)
    G = int(num_graphs)
    counts = np.bincount(batch, minlength=G)[:G].astype(np.int64)
    starts = np.concatenate([[0], np.cumsum(counts)[:-1]])
    tiles_g = (counts + P - 1) // P

    S = (G + n_cores - 1) // n_cores
    order = np.argsort(-tiles_g, kind="stable")
    core_graphs = [[] for _ in range(n_cores)]
    core_load = [0] * n_cores
    for g in order:
        cands = [c for c in range(n_cores) if len(core_graphs[c]) < S]
        c = min(cands, key=lambda cc: (core_load[cc], cc))
        core_graphs[c].append(int(g))
        core_load[c] += int(tiles_g[g])
    for c in range(n_cores):
        core_graphs[c].sort(key=lambda g: -int(tiles_g[g]))
        while len(core_graphs[c]) < S:
            core_graphs[c].append(-1)

    Ls = []
    for s in range(S):
        L = max(
            int(tiles_g[core_graphs[c][s]]) if core_graphs[c][s] >= 0 else 0
            for c in range(n_cores)
        )
        Ls.append(max(L, 1))
    return counts, starts, core_graphs, Ls


def _pack_inputs(x, counts, starts, core_graphs, Ls, n_cores):
    T = sum(Ls)
    slot_off = np.concatenate([[0], np.cumsum(Ls)[:-1]])
    xT = np.ascontiguousarray(
        np.transpose(x, (0, 2, 1)).astype(np.float16))  # [B, DIM, N] fp16
    per_core = []
    for c in range(n_cores):
        xTp = np.zeros((B, DIM, T * P), np.float16)
        qsc = np.zeros((T * P,), np.float32)
        kvm = np.zeros((T * P,), np.float32)
        for s, g in enumerate(core_graphs[c]):
            if g < 0 or counts[g] == 0:
                continue
            n0, ng = int(starts[g]), int(counts[g])
            off = int(slot_off[s]) * P
            xTp[:, :, off:off + ng] = xT[:, :, n0:n0 + ng]
            qsc[off:off + ng] = 1.0 / ng
            kvm[off:off + ng] = 1.0
        per_core.append((xTp, qsc, kvm))
    return per_core, slot_off


# ---------------------------------------------------------------------------
# device program
# ---------------------------------------------------------------------------

def _build_program(T, Ls, n_cores, ln_general, bo_zero=False, repeat=1):
    from contextlib import ExitStack

    nc = bacc.Bacc("TRN2", target_bir_lowering=False, debug=False,
                   num_devices=n_cores)

    xT = nc.dram_tensor("xT", [B, DIM, T * P], F16, kind="ExternalInput")
    wq = nc.dram_tensor("wqkvT", [DIM, R], F16, kind="ExternalInput")
    wo = nc.dram_tensor("woutT", [INNER, DIM], F16, kind="ExternalInput")
    bo = nc.dram_tensor("bout", [DIM], F32, kind="ExternalInput")
    qsc = nc.dram_tensor("qsc", [T * P], F32, kind="ExternalInput")
    kvm = nc.dram_tensor("kvm", [T * P], F32, kind="ExternalInput")
    if ln_general:
        lnp = nc.dram_tensor("lnp", [4, DH], F32, kind="ExternalInput")
    out = nc.dram_tensor("out", [B, T * P, DIM], F16, kind="ExternalOutput")

    slot_off = [0]
    for L in Ls[:-1]:
        slot_off.append(slot_off[-1] + L)

    Sqrt = mybir.ActivationFunctionType.Sqrt
    mult = mybir.AluOpType.mult
    add = mybir.AluOpType.add
    sub = mybir.AluOpType.subtract

    with ExitStack() as ctx:
        tc = ctx.enter_context(tile.TileContext(nc))
        const = ctx.enter_context(tc.tile_pool(name="const", bufs=1))

        WQ = const.tile([P, NCH, R], F16, tag="WQ")
        nc.sync.dma_start(out=WQ[:], in_=wq.ap().rearrange("(k c) r -> c k r", c=P))
        WO = const.tile([P, NCH, DIM], F16, tag="WO")
        nc.sync.dma_start(out=WO[:], in_=wo.ap().rearrange("(k c) d -> c k d", c=P))
        QS = const.tile([P, T], F32, tag="QS")
        nc.sync.dma_start(out=QS[:], in_=qsc.ap().rearrange("(t p) -> p t", p=P))
        KM = const.tile([P, T], F32, tag="KM")
        nc.sync.dma_start(out=KM[:], in_=kvm.ap().rearrange("(t p) -> p t", p=P))
        EPSC = const.tile([P, 1], F32, tag="EPSC")
        nc.vector.memset(EPSC[:], EPS)
        ONESP = const.tile([P, DH], F16, tag="ONESP")
        nc.vector.memset(ONESP[:], 1.0)
        if not bo_zero:
            BOt = const.tile([P, DIM], F32, tag="BO")
            nc.sync.dma_start(out=BOt[:], in_=bo.ap().partition_broadcast(P))
        if ln_general:
            LNP = const.tile([P, 4, DH], F32, tag="LNP")
            nc.sync.dma_start(out=LNP[:], in_=lnp.ap().partition_broadcast(P))

        xpool = ctx.enter_context(tc.tile_pool(name="xp", bufs=3))
        kvsb = ctx.enter_context(tc.tile_pool(name="kvsb", bufs=3))
        stat = ctx.enter_context(tc.tile_pool(name="stat", bufs=3))
        qstash = ctx.enter_context(tc.tile_pool(name="qstash", bufs=2 * NPAIRS))
        fixp = ctx.enter_context(tc.tile_pool(name="fixp", bufs=2))
        bdsb = ctx.enter_context(tc.tile_pool(name="bd", bufs=2))
        ohsb = ctx.enter_context(tc.tile_pool(name="oh", bufs=2 * NPAIRS))
        outsb = ctx.enter_context(tc.tile_pool(name="outsb", bufs=3))

        kvps = ctx.enter_context(tc.tile_pool(name="kvps", bufs=2, space="PSUM"))
        pmix = ctx.enter_context(tc.tile_pool(name="pmix", bufs=2, space="PSUM"))
        gps = ctx.enter_context(tc.tile_pool(name="gps", bufs=1, space="PSUM"))

        def _phase2(job):
            b2, soff2, grps2, qts2, bd2 = job
            for (gt0, gw) in grps2:
                GW = gw * P
                ohs = []
                for p in range(NPAIRS):
                    oh = pmix.tile([P, GW], F32, tag="pm")
                    nc.tensor.matmul(
                        oh[:], lhsT=bd2[:, p, :],
                        rhs=qts2[p][:, gt0 * P:gt0 * P + GW],
                        start=True, stop=True)
                    os_ = ohsb.tile([P, GW], F16, name="ohs", tag="oh")
                    nc.scalar.copy(out=os_[:], in_=oh[:])
                    ohs.append(os_)

                for tl in range(gw):
                    t_ = gt0 + tl
                    ti = soff2 + t_
                    ops = pmix.tile([P, DIM], F32, tag="pm")
                    for p in range(NPAIRS):
                        nc.tensor.matmul(
                            ops[:],
                            lhsT=ohs[p][:, tl * P:(tl + 1) * P],
                            rhs=WO[:, p, :],
                            start=(p == 0), stop=(p == NPAIRS - 1))
                    ot = outsb.tile([P, DIM], F16, tag="ot")
                    if bo_zero:
                        nc.scalar.mul(ot[:], ops[:], QS[:, ti:ti + 1])
                    else:
                        nc.vector.scalar_tensor_tensor(
                            ot[:], ops[:], QS[:, ti:ti + 1], BOt[:],
                            op0=mult, op1=add)
                    nc.sync.dma_start(
                        out=out.ap()[b2, ti * P:(ti + 1) * P, :],
                        in_=ot[:])

        prev_job = None
        for _rep in range(repeat):
          for b in range(B):
            for s, L in enumerate(Ls):
                soff = slot_off[s]
                # padded to [P, 4, 128] so each Gram owns a full 2KB
                # PSUM bank: matmul start=True clears has_written for the
                # WHOLE bank, so co-residents would corrupt each other.
                GEO = gps.tile([P, NPAIRS, P], F32, tag="geo")
                # Gram init via matmul start=True pending-zero marking:
                # the first matmul of each partition-region marks its full
                # 2KB bank row pending-zero (per-partition), so each of the
                # three region chains below needs exactly one start.
                qts = [qstash.tile([P, L * P], F16, name=f"qts{i}", tag="qstash")
                       for i in range(NPAIRS)]
                grps = _groups(L)

                # aug-Gram accumulate, software-pipelined one tile behind
                # the kv matmul so the in-order PE never stalls on the LN
                # chain of the tile it just produced.
                #   even head 2p: [65, 65] at GE[0:65, p, :]
                #   odd head 2p+1: k'-rows [64, 65] at GO[64:128, p, :]
                #                  gamma-row [1, 65] at GO[32:33, p, :]
                # (PE tile bases must be 0/32/64, so the odd 65-row Gram is
                #  split: main block at base 64 so the bd copy stays
                #  same-partition, aug row parked at 32.)
                first = [True]

                def _emit_ktv(XT_, YT_):
                    st = first[0]
                    first[0] = False
                    for p in range(NPAIRS):
                        nc.tensor.matmul(
                            GEO[0:DH, p, 0:DH],
                            lhsT=XT_[:, 2 * p, :], rhs=YT_[:, 2 * p, :],
                            start=(st and p == 0), stop=False,
                            skip_group_check=True)
                        nc.tensor.matmul(
                            GEO[DH:P, p, 0:DH],
                            lhsT=XT_[:, 2 * p + 1, :], rhs=YT_[:, 2 * p + 1, :],
                            start=(st and p == 0), stop=False,
                            skip_group_check=True)
                pend = [None]

                # ---- phase 1: qkv projection, LN stats, apply, aug-Gram ----
                for (gt0, gw) in grps:
                    GW = gw * P
                    n0 = (soff + gt0) * P

                    xt = xpool.tile([P, NCH, GW], F16, tag="xt")
                    nc.sync.dma_start(
                        out=xt[:],
                        in_=xT.ap()[b].rearrange("(k c) n -> c k n", c=P)[:, :, n0:n0 + GW],
                    )

                    # q^T pairs: stationary = W_q pair block, moving = x^T
                    for p in range(NPAIRS):
                        qtp = pmix.tile([P, GW], F32, tag="pm")
                        for k in range(NCH):
                            nc.tensor.matmul(
                                qtp[:],
                                lhsT=WQ[:, k, p * P:(p + 1) * P],
                                rhs=xt[:, k, :],
                                start=(k == 0), stop=(k == NCH - 1),
                            )
                        nc.scalar.copy(
                            out=qts[p][:, gt0 * P:gt0 * P + GW], in_=qtp[:])

                    for tl in range(gw):
                        t_ = gt0 + tl
                        ti = soff + t_

                        kv = kvps.tile([P, 2, HEADS, DH], F32, tag="kv")
                        for k in range(NCH):
                            lx = xt[:, k, tl * P:(tl + 1) * P]
                            nc.tensor.matmul(
                                kv[:, 0], lhsT=lx,
                                rhs=WQ[:, k, INNER:2 * INNER],
                                start=(k == 0), stop=(k == NCH - 1))
                            nc.tensor.matmul(
                                kv[:, 1], lhsT=lx,
                                rhs=WQ[:, k, 2 * INNER:R],
                                start=(k == 0), stop=(k == NCH - 1))
                        if pend[0] is not None:
                            _emit_ktv(*pend[0])
                            pend[0] = None

                        # Stage kv into SBUF fp16 (ACT) — frees the PSUM
                        # tile early and lets GPSIMD (SBUF-only) share the
                        # elementwise work. All LN math reads KVS.
                        KVS = kvsb.tile([P, 2, HEADS, DH], F16, tag="kvs")
                        nc.scalar.copy(out=KVS[:], in_=kv[:])

                        # LN stats: per-head sums and sums of squares via
                        # multi-group tensor_reduce (the wrapper keeps the
                        # window dim so head groups cannot merge).
                        SQ = kvsb.tile([P, 2, HEADS, DH], F16, tag="sq")
                        nc.vector.tensor_tensor(SQ[:], KVS[:], KVS[:], op=mult)
                        SU = stat.tile([P, 2, HEADS], F32, tag="su")
                        nc.vector.tensor_reduce(
                            SU[:], KVS[:], axis=mybir.AxisListType.X, op=add)
                        SS = stat.tile([P, 2, HEADS], F32, tag="ss")
                        nc.vector.tensor_reduce(
                            SS[:], SQ[:], axis=mybir.AxisListType.X, op=add)
                        # var*DH = SS - SU^2/DH ;  sd = sqrt(var + eps)
                        T1 = stat.tile([P, 2, HEADS], F32, tag="t1")
                        nc.gpsimd.tensor_tensor(T1[:], SU[:], SU[:], op=mult)
                        U2 = stat.tile([P, 2, HEADS], F32, tag="u2")
                        nc.vector.scalar_tensor_tensor(
                            U2[:], T1[:], -1.0 / DH, SS[:], op0=mult, op1=add)
                        SD = stat.tile([P, 2, HEADS], F32, tag="sd")
                        nc.scalar.activation(SD[:], U2[:], Sqrt,
                                             bias=EPSC[:, 0:1], scale=1.0 / DH)
                        RS = stat.tile([P, 2, HEADS], F32, tag="rs")
                        nc.vector.reciprocal(RS[:], SD[:])
                        # alpha/beta = rstd * mask ; gamma/delta = -mu * alpha
                        # ALGM layout [P, (al|gm), side, head]; fp16 copy for
                        # the applies / aug columns.
                        ALGM = stat.tile([P, 2, 2, HEADS], F32, tag="algm")
                        nc.vector.tensor_scalar(ALGM[:, 0], RS[:],
                                                KM[:, ti:ti + 1], None, op0=mult)
                        nc.vector.scalar_tensor_tensor(
                            ALGM[:, 1], SU[:], -1.0 / DH, ALGM[:, 0],
                            op0=mult, op1=mult)
                        AG16 = stat.tile([P, 2, 2, HEADS], F16, tag="ag16")
                        nc.gpsimd.tensor_copy(AG16[:], ALGM[:])

                        # apply: full LN per side as broadcast mult + add
                        # (alpha*k + gamma), plain [P, 8, 64] layout.
                        XT_ = kvsb.tile([P, HEADS, DH], F16, tag="xtl")
                        YT_ = kvsb.tile([P, HEADS, DH], F16, tag="ytl")
                        for side, dst, eng in ((0, XT_, nc.vector),
                                               (1, YT_, nc.gpsimd)):
                            srcv = _apv(KVS, 0, P, side * INNER,
                                        [[DH, HEADS], [1, DH]])
                            alv = _apv(AG16, 0, P, side * HEADS,
                                       [[1, HEADS], [0, DH]])
                            gmv = _apv(AG16, 0, P, 2 * HEADS + side * HEADS,
                                       [[1, HEADS], [0, DH]])
                            eng.tensor_tensor(dst[:], srcv, alv, op=mult)
                            eng.tensor_tensor(dst[:], dst[:], gmv, op=add)

                        if ln_general:
                            # z = w*(alpha*k + gamma) + b*mask  (slow, correct)
                            for side, dst, eng2 in ((0, XT_, nc.vector),
                                                    (1, YT_, nc.vector)):
                                wv = _apv(LNP, 0, P, 2 * side * DH,
                                          [[0, HEADS], [1, DH]])
                                eng2.tensor_tensor(dst[:], dst[:], wv, op=mult)
                                BM = stat.tile([P, DH], F32, tag="bm")
                                nc.vector.tensor_scalar(
                                    BM[:], LNP[:, 2 * side + 1], KM[:, ti:ti + 1],
                                    None, op0=mult)
                                bmbc = _apv(BM, 0, P, 0, [[0, HEADS], [1, DH]])
                                eng2.tensor_tensor(dst[:], dst[:], bmbc, op=add)

                        pend[0] = (XT_, YT_)

                if pend[0] is not None:
                    _emit_ktv(*pend[0])
                    pend[0] = None

                # previous slot's phase 2 fills the PE while this slot's
                # Gram fixup waits on the tail of the LN chain.
                if prev_job is not None:
                    _phase2(prev_job)
                    prev_job = None

                # ---- block-diagonal ktv for the pair-batched phase 2 ----
                bd = bdsb.tile([P, NPAIRS, P], F16, tag="bd")
                nc.gpsimd.memset(bd[:], 0.0)
                nc.vector.tensor_copy(bd[0:DH, :, 0:DH], GEO[0:DH, :, 0:DH])
                nc.vector.tensor_copy(bd[DH:P, :, DH:P], GEO[DH:P, :, 0:DH])

                prev_job = (b, soff, grps, qts, bd)

        if prev_job is not None:
            _phase2(prev_job)

    nc.compile()
    return nc


# ---------------------------------------------------------------------------
# entry point
# ---------------------------------------------------------------------------

def _run(x, w_qkv, ln1_w, ln1_b, ln2_w, ln2_b, w_out, b_out, batch,
         num_graphs, n_cores=N_CORES, trace=False):
    x = np.asarray(x)
    counts, starts, core_graphs, Ls = _plan(batch, num_graphs, n_cores)
    per_core, slot_off = _pack_inputs(x, counts, starts, core_graphs, Ls, n_cores)
    T = sum(Ls)

    ln1_w = np.asarray(ln1_w, np.float32)
    ln1_b = np.asarray(ln1_b, np.float32)
    ln2_w = np.asarray(ln2_w, np.float32)
    ln2_b = np.asarray(ln2_b, np.float32)
    ln_general = not (
        np.all(ln1_w == 1.0) and np.all(ln1_b == 0.0)
        and np.all(ln2_w == 1.0) and np.all(ln2_b == 0.0)
    )

    bout_np = np.asarray(b_out, np.float32)
    bo_zero = bool(np.all(bout_np == 0.0))
    key = (T, tuple(Ls), n_cores, ln_general, bo_zero)
    nc = _PROGRAM_CACHE.get(key)
    if nc is None:
        nc = _build_program(T, tuple(Ls), n_cores, ln_general, bo_zero)
        _PROGRAM_CACHE[key] = nc

    wqkvT = np.ascontiguousarray(np.asarray(w_qkv, np.float32).T.astype(np.float16))
    woutT = np.ascontiguousarray(np.asarray(w_out, np.float32).T.astype(np.float16))
    bout = np.ascontiguousarray(bout_np)
    lnp = np.stack([ln1_w, ln1_b, ln2_w, ln2_b])

    in_maps = []
    for c in range(n_cores):
        xTp, qscv, kvmv = per_core[c]
        m = {"xT": xTp, "wqkvT": wqkvT, "woutT": woutT, "bout": bout,
             "qsc": qscv, "kvm": kvmv}
        if ln_general:
            m["lnp"] = lnp
        in_maps.append(m)

    res = run_bass_kernel_spmd(nc, in_maps, list(range(n_cores)), trace=trace)

    N = x.shape[1]
    y = np.empty((B, N, DIM), np.float32)
    for c in range(n_cores):
        oc = res.results[c]["out"]
        for s, g in enumerate(core_graphs[c]):
            if g < 0 or counts[g] == 0:
                continue
            n0, ng = int(starts[g]), int(counts[g])
            off = int(slot_off[s]) * P
            y[:, n0:n0 + ng, :] = oc[:, off:off + ng, :].astype(np.float32)
    return y, res


def kernel(**inputs):
    trace = os.environ.get("GALERKIN_TRACE") == "1"
    y, _ = _run(
        inputs["x"], inputs["w_qkv"], inputs["ln1_w"], inputs["ln1_b"],
        inputs["ln2_w"], inputs["ln2_b"], inputs["w_out"], inputs["b_out"],
        inputs["batch"], inputs["num_graphs"], trace=trace,
    )
    return y


# revision 6
# speedup vs baseline: 4.5463x; 1.0116x over previous
"""Galerkin attention (ragged graph segments) on 8 Trainium2 NeuronCores — v2.

Math (per reference):
  qkv = x @ w_qkv.T ; split q,k,v -> [B, H, N, DH]
  k, v  <- LayerNorm over DH (eps=1e-6, affine)
  per graph g (sorted contiguous segments of N): ktv[g] = k_g^T v_g
  out_n = (q_n / size(g(n))) @ ktv[g(n)]
  y = out @ w_out.T + b_out

v2 design vs the fp32r baseline:
  * fp16 data path (x, W_qkv, k'/v', bd) -> 1 cycle/row matmuls at ALL free
    sizes, halved input DMA; all accumulation stays fp32 in PSUM.
  * LayerNorm stats via an ACT fp16 staging copy of kv plus two
    multi-group tensor_reduce calls (sum, sum of squares) and a handful of
    fused [P,16] ops instead of 16 per-head bn_stats/bn_aggr pairs; the
    elementwise work is spread over DVE + GPSIMD (GPSIMD is SBUF-only).
  * LN apply as two broadcast ops per side (alpha*k + gamma via stride-0
    free-dim APs); per-head 64x64 Grams pair-pack into one PSUM bank with
    even heads at partitions 0:64 and odd heads at 64:128 (matmul output
    base picks the partition block), so the block-diagonal bd assembles
    with two batched same-partition copies and no fixup pass.
  * Phase 2 of each slot is emitted between the next slot's phase 1 and
    its Gram fixup, and the Gram matmuls trail the kv projection by one
    tile, so the in-order PE never stalls on the LayerNorm chain.
"""

import os
import sys

if "/opt/trn_rl_repo" not in sys.path:
    sys.path.insert(0, "/opt/trn_rl_repo")

import numpy as np

import concourse.bacc as bacc
import concourse.bass as bass
import concourse.mybir as mybir
import concourse.tile as tile
from concourse.bass_utils import run_bass_kernel_spmd

P = 128
B = 2
DIM = 512
HEADS = 8
DH = 64
DA = DH + 1                 # augmented head dim
INNER = HEADS * DH          # 512
R = 3 * INNER               # 1536
NCH = DIM // P              # 4 contraction chunks
NPAIRS = HEADS // 2         # 4 head pairs
EPS = 1e-6
N_CORES = 8
F32 = mybir.dt.float32
F32R = mybir.dt.float32r
F16 = mybir.dt.float16

_PROGRAM_CACHE: dict = {}


def _r(ap):
    return ap.bitcast(F32R)


def _apv(t, poff, pcnt, foff, dims):
    """Manual AP view of tile t: partition range [poff, poff+pcnt), free
    offset foff, free dims [[stride, n], ...] (strides in elements)."""
    a = t[:]
    ps = a.ap[0][0]
    return bass.AP(a.tensor, a.offset + poff * ps + foff,
                   [[ps, pcnt]] + [list(d) for d in dims])


def _groups(L, grp=4):
    """Split L tiles into chunks of <= grp, avoiding a trailing chunk of 1
    (which would drop fp32r matmuls below the 256 free-dim full-speed gate)."""
    if L <= grp:
        return [(0, L)]
    out = []
    t0 = 0
    while L - t0 > grp + 1:
        out.append((t0, grp))
        t0 += grp
    rem = L - t0
    if rem > grp:  # rem == grp + 1
        out.append((t0, grp - 1))
        out.append((t0 + grp - 1, 2))
    else:
        out.append((t0, rem))
    return out


# ---------------------------------------------------------------------------
# host-side planning (unchanged from baseline)
# ---------------------------------------------------------------------------

def _plan(batch, num_graphs, n_cores):
    batch = np.asarray(batch).astype(np.int64)
    G = int(num_graphs)
    counts = np.bincount(batch, minlength=G)[:G].astype(np.int64)
    starts = np.concatenate([[0], np.cumsum(counts)[:-1]])
    tiles_g = (counts + P - 1) // P

    S = (G + n_cores - 1) // n_cores
    order = np.argsort(-tiles_g, kind="stable")
    core_graphs = [[] for _ in range(n_cores)]
    core_load = [0] * n_cores
    for g in order:
        cands = [c for c in range(n_cores) if len(core_graphs[c]) < S]
        c = min(cands, key=lambda cc: (core_load[cc], cc))
        core_graphs[c].append(int(g))
        core_load[c] += int(tiles_g[g])
    for c in range(n_cores):
        core_graphs[c].sort(key=lambda g: -int(tiles_g[g]))
        while len(core_graphs[c]) < S:
            core_graphs[c].append(-1)

    Ls = []
    for s in range(S):
        L = max(
            int(tiles_g[core_graphs[c][s]]) if core_graphs[c][s] >= 0 else 0
            for c in range(n_cores)
        )
        Ls.append(max(L, 1))
    return counts, starts, core_graphs, Ls


def _pack_inputs(x, counts, starts, core_graphs, Ls, n_cores):
    T = sum(Ls)
    slot_off = np.concatenate([[0], np.cumsum(Ls)[:-1]])
    xT = np.ascontiguousarray(
        np.transpose(x, (0, 2, 1)).astype(np.float16))  # [B, DIM, N] fp16
    per_core = []
    for c in range(n_cores):
        xTp = np.zeros((B, DIM, T * P), np.float16)
        qsc = np.zeros((T * P,), np.float32)
        kvm = np.zeros((T * P,), np.float32)
        for s, g in enumerate(core_graphs[c]):
            if g < 0 or counts[g] == 0:
                continue
            n0, ng = int(starts[g]), int(counts[g])
            off = int(slot_off[s]) * P
            xTp[:, :, off:off + ng] = xT[:, :, n0:n0 + ng]
            qsc[off:off + ng] = 1.0 / ng
            kvm[off:off + ng] = 1.0
        per_core.append((xTp, qsc, kvm))
    return per_core, slot_off


# ---------------------------------------------------------------------------
# device program
# ---------------------------------------------------------------------------

def _build_program(T, Ls, n_cores, ln_general, bo_zero=False, repeat=1):
    from contextlib import ExitStack

    nc = bacc.Bacc("TRN2", target_bir_lowering=False, debug=False,
                   num_devices=n_cores)

    xT = nc.dram_tensor("xT", [B, DIM, T * P], F16, kind="ExternalInput")
    wq = nc.dram_tensor("wqkvT", [DIM, R], F16, kind="ExternalInput")
    wo = nc.dram_tensor("woutT", [INNER, DIM], F16, kind="ExternalInput")
    bo = nc.dram_tensor("bout", [DIM], F32, kind="ExternalInput")
    qsc = nc.dram_tensor("qsc", [T * P], F32, kind="ExternalInput")
    kvm = nc.dram_tensor("kvm", [T * P], F32, kind="ExternalInput")
    if ln_general:
        lnp = nc.dram_tensor("lnp", [4, DH], F32, kind="ExternalInput")
    out = nc.dram_tensor("out", [B, T * P, DIM], F16, kind="ExternalOutput")

    slot_off = [0]
    for L in Ls[:-1]:
        slot_off.append(slot_off[-1] + L)

    Sqrt = mybir.ActivationFunctionType.Sqrt
    mult = mybir.AluOpType.mult
    add = mybir.AluOpType.add
    sub = mybir.AluOpType.subtract

    with ExitStack() as ctx:
        tc = ctx.enter_context(tile.TileContext(nc))
        const = ctx.enter_context(tc.tile_pool(name="const", bufs=1))

        WQ = const.tile([P, NCH, R], F16, tag="WQ")
        nc.sync.dma_start(out=WQ[:], in_=wq.ap().rearrange("(k c) r -> c k r", c=P))
        WO = const.tile([P, NCH, DIM], F16, tag="WO")
        nc.sync.dma_start(out=WO[:], in_=wo.ap().rearrange("(k c) d -> c k d", c=P))
        QS = const.tile([P, T], F32, tag="QS")
        nc.sync.dma_start(out=QS[:], in_=qsc.ap().rearrange("(t p) -> p t", p=P))
        KM = const.tile([P, T], F32, tag="KM")
        nc.sync.dma_start(out=KM[:], in_=kvm.ap().rearrange("(t p) -> p t", p=P))
        EPSC = const.tile([P, 1], F32, tag="EPSC")
        nc.vector.memset(EPSC[:], EPS)
        ONESP = const.tile([P, DH], F16, tag="ONESP")
        nc.vector.memset(ONESP[:], 1.0)
        if not bo_zero:
            BOt = const.tile([P, DIM], F32, tag="BO")
            nc.sync.dma_start(out=BOt[:], in_=bo.ap().partition_broadcast(P))
        if ln_general:
            LNP = const.tile([P, 4, DH], F32, tag="LNP")
            nc.sync.dma_start(out=LNP[:], in_=lnp.ap().partition_broadcast(P))

        xpool = ctx.enter_context(tc.tile_pool(name="xp", bufs=3))
        kvsb = ctx.enter_context(tc.tile_pool(name="kvsb", bufs=3))
        stat = ctx.enter_context(tc.tile_pool(name="stat", bufs=3))
        qstash = ctx.enter_context(tc.tile_pool(name="qstash", bufs=2 * NPAIRS))
        fixp = ctx.enter_context(tc.tile_pool(name="fixp", bufs=2))
        bdsb = ctx.enter_context(tc.tile_pool(name="bd", bufs=2))
        ohsb = ctx.enter_context(tc.tile_pool(name="oh", bufs=2 * NPAIRS))
        outsb = ctx.enter_context(tc.tile_pool(name="outsb", bufs=3))

        kvps = ctx.enter_context(tc.tile_pool(name="kvps", bufs=2, space="PSUM"))
        pmix = ctx.enter_context(tc.tile_pool(name="pmix", bufs=2, space="PSUM"))
        gps = ctx.enter_context(tc.tile_pool(name="gps", bufs=1, space="PSUM"))

        def _phase2(job):
            b2, soff2, grps2, qts2, bd2 = job
            for (gt0, gw) in grps2:
                GW = gw * P
                ohs = []
                for p in range(NPAIRS):
                    oh = pmix.tile([P, GW], F32, tag="pm")
                    nc.tensor.matmul(
                        oh[:], lhsT=bd2[:, p, :],
                        rhs=qts2[p][:, gt0 * P:gt0 * P + GW],
                        start=True, stop=True)
                    os_ = ohsb.tile([P, GW], F16, name="ohs", tag="oh")
                    nc.scalar.copy(out=os_[:], in_=oh[:])
                    ohs.append(os_)

                for tl in range(gw):
                    t_ = gt0 + tl
                    ti = soff2 + t_
                    ops = pmix.tile([P, DIM], F32, tag="pm")
                    for p in range(NPAIRS):
                        nc.tensor.matmul(
                            ops[:],
                            lhsT=ohs[p][:, tl * P:(tl + 1) * P],
                            rhs=WO[:, p, :],
                            start=(p == 0), stop=(p == NPAIRS - 1))
                    ot = outsb.tile([P, DIM], F16, tag="ot")
                    if bo_zero:
                        nc.scalar.mul(ot[:], ops[:], QS[:, ti:ti + 1])
                    else:
                        nc.vector.scalar_tensor_tensor(
                            ot[:], ops[:], QS[:, ti:ti + 1], BOt[:],
                            op0=mult, op1=add)
                    nc.sync.dma_start(
                        out=out.ap()[b2, ti * P:(ti + 1) * P, :],
                        in_=ot[:])

        prev_job = None
        for _rep in range(repeat):
          for b in range(B):
            for s, L in enumerate(Ls):
                soff = slot_off[s]
                # padded to [P, 4, 128] so each Gram owns a full 2KB
                # PSUM bank: matmul start=True clears has_written for the
                # WHOLE bank, so co-residents would corrupt each other.
                GEO = gps.tile([P, NPAIRS, P], F32, tag="geo")
                # Gram init via matmul start=True pending-zero marking:
                # the first matmul of each partition-region marks its full
                # 2KB bank row pending-zero (per-partition), so each of the
                # three region chains below needs exactly one start.
                qts = [qstash.tile([P, L * P], F16, name=f"qts{i}", tag="qstash")
                       for i in range(NPAIRS)]
                grps = _groups(L)

                # aug-Gram accumulate, software-pipelined one tile behind
                # the kv matmul so the in-order PE never stalls on the LN
                # chain of the tile it just produced.
                #   even head 2p: [65, 65] at GE[0:65, p, :]
                #   odd head 2p+1: k'-rows [64, 65] at GO[64:128, p, :]
                #                  gamma-row [1, 65] at GO[32:33, p, :]
                # (PE tile bases must be 0/32/64, so the odd 65-row Gram is
                #  split: main block at base 64 so the bd copy stays
                #  same-partition, aug row parked at 32.)
                first = [True]

                def _emit_ktv(XT_, YT_):
                    st = first[0]
                    first[0] = False
                    for p in range(NPAIRS):
                        nc.tensor.matmul(
                            GEO[0:DH, p, 0:DH],
                            lhsT=XT_[:, 2 * p, :], rhs=YT_[:, 2 * p, :],
                            start=(st and p == 0), stop=False,
                            skip_group_check=True)
                        nc.tensor.matmul(
                            GEO[DH:P, p, 0:DH],
                            lhsT=XT_[:, 2 * p + 1, :], rhs=YT_[:, 2 * p + 1, :],
                            start=(st and p == 0), stop=False,
                            skip_group_check=True)
                pend = [None]

                # ---- phase 1: qkv projection, LN stats, apply, aug-Gram ----
                for (gt0, gw) in grps:
                    GW = gw * P
                    n0 = (soff + gt0) * P

                    xt = xpool.tile([P, NCH, GW], F16, tag="xt")
                    nc.sync.dma_start(
                        out=xt[:],
                        in_=xT.ap()[b].rearrange("(k c) n -> c k n", c=P)[:, :, n0:n0 + GW],
                    )

                    # q^T pairs: stationary = W_q pair block, moving = x^T
                    for p in range(NPAIRS):
                        qtp = pmix.tile([P, GW], F32, tag="pm")
                        for k in range(NCH):
                            nc.tensor.matmul(
                                qtp[:],
                                lhsT=WQ[:, k, p * P:(p + 1) * P],
                                rhs=xt[:, k, :],
                                start=(k == 0), stop=(k == NCH - 1),
                            )
                        nc.scalar.copy(
                            out=qts[p][:, gt0 * P:gt0 * P + GW], in_=qtp[:])

                    for tl in range(gw):
                        t_ = gt0 + tl
                        ti = soff + t_

                        kv = kvps.tile([P, 2, HEADS, DH], F32, tag="kv")
                        for k in range(NCH):
                            lx = xt[:, k, tl * P:(tl + 1) * P]
                            nc.tensor.matmul(
                                kv[:, 0], lhsT=lx,
                                rhs=WQ[:, k, INNER:2 * INNER],
                                start=(k == 0), stop=(k == NCH - 1))
                            nc.tensor.matmul(
                                kv[:, 1], lhsT=lx,
                                rhs=WQ[:, k, 2 * INNER:R],
                                start=(k == 0), stop=(k == NCH - 1))
                        if pend[0] is not None:
                            _emit_ktv(*pend[0])
                            pend[0] = None

                        # Stage kv into SBUF fp16 (ACT) — frees the PSUM
                        # tile early and lets GPSIMD (SBUF-only) share the
                        # elementwise work. All LN math reads KVS.
                        KVS = kvsb.tile([P, 2, HEADS, DH], F16, tag="kvs")
                        nc.scalar.copy(out=KVS[:], in_=kv[:])

                        # LN stats: per-head sums and sums of squares via
                        # multi-group tensor_reduce (the wrapper keeps the
                        # window dim so head groups cannot merge).
                        SQ = kvsb.tile([P, 2, HEADS, DH], F16, tag="sq")
                        nc.vector.tensor_tensor(SQ[:], KVS[:], KVS[:], op=mult)
                        SU = stat.tile([P, 2, HEADS], F32, tag="su")
                        nc.vector.tensor_reduce(
                            SU[:], KVS[:], axis=mybir.AxisListType.X, op=add)
                        SS = stat.tile([P, 2, HEADS], F32, tag="ss")
                        nc.vector.tensor_reduce(
                            SS[:], SQ[:], axis=mybir.AxisListType.X, op=add)
                        # var*DH = SS - SU^2/DH ;  sd = sqrt(var + eps)
                        T1 = stat.tile([P, 2, HEADS], F32, tag="t1")
                        nc.gpsimd.tensor_tensor(T1[:], SU[:], SU[:], op=mult)
                        U2 = stat.tile([P, 2, HEADS], F32, tag="u2")
                        nc.vector.scalar_tensor_tensor(
                            U2[:], T1[:], -1.0 / DH, SS[:], op0=mult, op1=add)
                        SD = stat.tile([P, 2, HEADS], F32, tag="sd")
                        nc.scalar.activation(SD[:], U2[:], Sqrt,
                                             bias=EPSC[:, 0:1], scale=1.0 / DH)
                        # alpha/beta = rstd ; gamma/delta = -mu * alpha.
                        # No pad-row mask needed: pad columns of x are exact
                        # zeros, so kv=0 -> mu=0 -> X = 0*rstd + 0 already
                        # (the general path masks its bias separately).
                        # ALGM layout [P, (al|gm), side, head].
                        ALGM = stat.tile([P, 2, 2, HEADS], F32, tag="algm")
                        nc.vector.reciprocal(ALGM[:, 0], SD[:])
                        nc.vector.scalar_tensor_tensor(
                            ALGM[:, 1], SU[:], -1.0 / DH, ALGM[:, 0],
                            op0=mult, op1=mult)
                        AG16 = stat.tile([P, 2, 2, HEADS], F16, tag="ag16")
                        nc.gpsimd.tensor_copy(AG16[:], ALGM[:])

                        # apply: full LN per side as broadcast mult + add
                        # (alpha*k + gamma), plain [P, 8, 64] layout.
                        XT_ = kvsb.tile([P, HEADS, DH], F16, tag="xtl")
                        YT_ = kvsb.tile([P, HEADS, DH], F16, tag="ytl")
                        for side, dst, eng in ((0, XT_, nc.vector),
                                               (1, YT_, nc.gpsimd)):
                            srcv = _apv(KVS, 0, P, side * INNER,
                                        [[DH, HEADS], [1, DH]])
                            alv = _apv(AG16, 0, P, side * HEADS,
                                       [[1, HEADS], [0, DH]])
                            gmv = _apv(AG16, 0, P, 2 * HEADS + side * HEADS,
                                       [[1, HEADS], [0, DH]])
                            eng.tensor_tensor(dst[:], srcv, alv, op=mult)
                            eng.tensor_tensor(dst[:], dst[:], gmv, op=add)

                        if ln_general:
                            # z = w*(alpha*k + gamma) + b*mask  (slow, correct)
                            for side, dst, eng2 in ((0, XT_, nc.vector),
                                                    (1, YT_, nc.vector)):
                                wv = _apv(LNP, 0, P, 2 * side * DH,
                                          [[0, HEADS], [1, DH]])
                                eng2.tensor_tensor(dst[:], dst[:], wv, op=mult)
                                BM = stat.tile([P, DH], F32, tag="bm")
                                nc.vector.tensor_scalar(
                                    BM[:], LNP[:, 2 * side + 1], KM[:, ti:ti + 1],
                                    None, op0=mult)
                                bmbc = _apv(BM, 0, P, 0, [[0, HEADS], [1, DH]])
                                eng2.tensor_tensor(dst[:], dst[:], bmbc, op=add)

                        pend[0] = (XT_, YT_)

                if pend[0] is not None:
                    _emit_ktv(*pend[0])
                    pend[0] = None

                # previous slot's phase 2 fills the PE while this slot's
                # Gram fixup waits on the tail of the LN chain.
                if prev_job is not None:
                    _phase2(prev_job)
                    prev_job = None

                # ---- block-diagonal ktv for the pair-batched phase 2 ----
                bd = bdsb.tile([P, NPAIRS, P], F16, tag="bd")
                nc.gpsimd.memset(bd[:], 0.0)
                nc.vector.tensor_copy(bd[0:DH, :, 0:DH], GEO[0:DH, :, 0:DH])
                nc.vector.tensor_copy(bd[DH:P, :, DH:P], GEO[DH:P, :, 0:DH])

                prev_job = (b, soff, grps, qts, bd)

        if prev_job is not None:
            _phase2(prev_job)

    nc.compile()
    return nc


# ---------------------------------------------------------------------------
# entry point
# ---------------------------------------------------------------------------

def _run(x, w_qkv, ln1_w, ln1_b, ln2_w, ln2_b, w_out, b_out, batch,
         num_graphs, n_cores=N_CORES, trace=False):
    x = np.asarray(x)
    counts, starts, core_graphs, Ls = _plan(batch, num_graphs, n_cores)
    per_core, slot_off = _pack_inputs(x, counts, starts, core_graphs, Ls, n_cores)
    T = sum(Ls)

    ln1_w = np.asarray(ln1_w, np.float32)
    ln1_b = np.asarray(ln1_b, np.float32)
    ln2_w = np.asarray(ln2_w, np.float32)
    ln2_b = np.asarray(ln2_b, np.float32)
    ln_general = not (
        np.all(ln1_w == 1.0) and np.all(ln1_b == 0.0)
        and np.all(ln2_w == 1.0) and np.all(ln2_b == 0.0)
    )

    bout_np = np.asarray(b_out, np.float32)
    bo_zero = bool(np.all(bout_np == 0.0))
    key = (T, tuple(Ls), n_cores, ln_general, bo_zero)
    nc = _PROGRAM_CACHE.get(key)
    if nc is None:
        nc = _build_program(T, tuple(Ls), n_cores, ln_general, bo_zero)
        _PROGRAM_CACHE[key] = nc

    wqkvT = np.ascontiguousarray(np.asarray(w_qkv, np.float32).T.astype(np.float16))
    woutT = np.ascontiguousarray(np.asarray(w_out, np.float32).T.astype(np.float16))
    bout = np.ascontiguousarray(bout_np)
    lnp = np.stack([ln1_w, ln1_b, ln2_w, ln2_b])

    in_maps = []
    for c in range(n_cores):
        xTp, qscv, kvmv = per_core[c]
        m = {"xT": xTp, "wqkvT": wqkvT, "woutT": woutT, "bout": bout,
             "qsc": qscv, "kvm": kvmv}
        if ln_general:
            m["lnp"] = lnp
        in_maps.append(m)

    res = run_bass_kernel_spmd(nc, in_maps, list(range(n_cores)), trace=trace)

    N = x.shape[1]
    y = np.empty((B, N, DIM), np.float32)
    for c in range(n_cores):
        oc = res.results[c]["out"]
        for s, g in enumerate(core_graphs[c]):
            if g < 0 or counts[g] == 0:
                continue
            n0, ng = int(starts[g]), int(counts[g])
            off = int(slot_off[s]) * P
            y[:, n0:n0 + ng, :] = oc[:, off:off + ng, :].astype(np.float32)
    return y, res


def kernel(**inputs):
    trace = os.environ.get("GALERKIN_TRACE") == "1"
    y, _ = _run(
        inputs["x"], inputs["w_qkv"], inputs["ln1_w"], inputs["ln1_b"],
        inputs["ln2_w"], inputs["ln2_b"], inputs["w_out"], inputs["b_out"],
        inputs["batch"], inputs["num_graphs"], trace=trace,
    )
    return y
